# revision 18
# baseline (speedup 1.0000x reference)
"""Bass/Tile TRN2 kernel for per-model-batched causal self-attention.

Problem: x[M,B,S,D], qkv_w[M,D,3D], proj_w[M,D,D] -> out[M,B,S,D]
M=8 models sharded across 8 NeuronCores (embarrassingly parallel).

Per-core design (model m):
  xT      = PE-transpose(x_b)                       [D,S] f32
  qkT     = wqkv[:, :1024].T-proj via fp32r matmul  [1024,S] f32 (q^T,k^T rows)
  V       = x @ wqkv[:, 1024:]  (natural)           [S,512] -> bf16, +ones col
  st[k,q] = K @ Q^T  (fp32r, causal-trimmed)        PSUM f32
  p       = exp(st/8)  (ScalarE, bf16 out), diag blocks masked by tri01 mul
  y_aug   = p.T @ V_aug (bf16)  -> y[q,d] + softmax sums in col 64 (PSUM)
  y       = y_aug * (1/sums)  per-partition scalar  [S,D]
  ynT     = PE-transpose(y)                          [D,S]
  out     = ynT.T @ wproj (fp32r or bf16)
"""

import sys

if "/opt/trn_rl_repo" not in sys.path:
    sys.path.insert(0, "/opt/trn_rl_repo")

import numpy as np

import concourse.bass as bass
import concourse.mybir as mybir
import concourse.tile as tile
from concourse import bacc, bass_utils
from concourse.masks import make_identity, make_upper_triangular

M, B, S, D, H = 8, 4, 512, 512, 8
HD = D // H  # 64
F32 = mybir.dt.float32
F32R = mybir.dt.float32r
BF16 = mybir.dt.bfloat16

# --- knobs ---
PROJ_F32 = True  # final projection in fp32r (True) or bf16 (False)
N_CORES = 8

_cache = {}


def _r(ap):
    return ap.bitcast(F32R)


def build_nc(reps=1):
    nc = bacc.Bacc("TRN2", target_bir_lowering=False, debug=False)

    x_d = nc.dram_tensor("x", [B, S, D], F32, kind="ExternalInput")
    wqkv_d = nc.dram_tensor("wqkv", [D, 3 * D], F32, kind="ExternalInput")
    wproj_d = nc.dram_tensor("wproj", [D, D], F32, kind="ExternalInput")
    out_d = nc.dram_tensor("out", [B, S, D], F32, kind="ExternalOutput")

    pdt = F32R if PROJ_F32 else BF16
    ytdt = F32R if PROJ_F32 else BF16

    from contextlib import ExitStack, nullcontext

    with tile.TileContext(nc) as tc:
        with (
            tc.tile_pool(name="singles", bufs=1) as singles,
            tc.tile_pool(name="xp", bufs=2) as xpool,
            tc.tile_pool(name="xtp", bufs=2) as xtpool,
            tc.tile_pool(name="qk", bufs=2) as qkpool,
            tc.tile_pool(name="vp", bufs=2) as vpool,
            tc.tile_pool(name="se", bufs=2) as sepool,
            tc.tile_pool(name="yp", bufs=2) as ypool,
            tc.tile_pool(name="ytp", bufs=2) as ytpool,
            tc.tile_pool(name="op", bufs=3) as opool,
            tc.tile_pool(name="rp", bufs=2) as rpool,
            tc.tile_pool(name="ps_mm", bufs=2, space=bass.MemorySpace.PSUM) as ps_mm,
            tc.tile_pool(name="ps_att", bufs=3, space=bass.MemorySpace.PSUM) as ps_att,
        ):
          with tc.For_i(0, reps, 1) if reps > 1 else nullcontext():
            # ---- constants & weights (once) ----
            ident = singles.tile([128, 128], F32)
            make_identity(nc, ident[:])
            ident_r = singles.tile([128, 128], F32R)
            nc.vector.tensor_copy(out=ident_r[:], in_=ident[:])
            ident_y = ident_r
            if not PROJ_F32:
                ident_y = singles.tile([128, 128], BF16, name="identb")
                nc.vector.tensor_copy(out=ident_y[:], in_=ident[:])
            tri2 = singles.tile([128, 2, 128], BF16)  # upper-tri(incl diag) keep mask, x2
            make_upper_triangular(nc, tri2[:, 0, :], val=1.0, diag=True)
            nc.gpsimd.tensor_copy(out=tri2[:, 1, :], in_=tri2[:, 0, :])

            wqkv = singles.tile([128, 4, 3 * D], F32R)
            wproj = singles.tile([128, 4, D], F32R if PROJ_F32 else F32)
            wproj_c = wproj

            for b in range(B):
                # ---- load x_b ----
                x_sb = xpool.tile([128, 4, D], F32R, tag="x")  # [p, s_tile, d]
                nc.sync.dma_start(
                    out=x_sb[:], in_=x_d.ap().bitcast(F32R)[b].rearrange("(t p) d -> p t d", p=128)
                )
                if b == 0:
                    for dc in range(4):
                        nc.sync.dma_start(
                            out=wqkv[:, dc, :],
                            in_=wqkv_d.ap().bitcast(F32R)[dc * 128 : (dc + 1) * 128, :],
                        )
                    nc.sync.dma_start(
                        out=wproj[:],
                        in_=wproj_d.ap()
                        .bitcast(F32R if PROJ_F32 else F32)
                        .rearrange("(c p) o -> p c o", p=128),
                    )
                    if not PROJ_F32:
                        wproj_cc = singles.tile([128, 4, D], BF16, name="wprojc")
                        nc.vector.tensor_copy(out=wproj_cc[:], in_=wproj[:])
                        wproj_c = wproj_cc

                # ---- xT[d, s] via PE transpose ----
                xT = []
                for dc in range(4):
                    tp = ps_mm.tile([128, 512], F32, tag="mm")
                    for st in range(4):
                        nc.tensor.transpose(
                            tp[:, st * 128 : (st + 1) * 128].bitcast(F32R),
                            x_sb[:, st, dc * 128 : (dc + 1) * 128],
                            ident_r[:],
                        )
                    xt = xtpool.tile([128, 512], F32R, tag=f"xt{dc}")
                    nc.vector.tensor_copy(out=xt[:], in_=tp[:])
                    xT.append(xt)

                # ---- qkT[o, s] = wqkv[:, :1024].T @ xT  (fp32r) ----
                qkT = []
                for mt in range(8):
                    mp = ps_mm.tile([128, 512], F32, tag="mm")
                    for dc in range(4):
                        nc.tensor.matmul(
                            mp[:],
                            wqkv[:, dc, mt * 128 : (mt + 1) * 128],
                            xT[dc][:],
                            start=(dc == 0),
                            stop=(dc == 3),
                        )
                    qk = qkpool.tile([128, 512], F32R, tag=f"qk{mt}")
                    nc.vector.tensor_copy(out=qk[:], in_=mp[:])
                    qkT.append(qk)

                # ---- V[s, o'] natural (fp32r) + ones col -> bf16 V_aug ----
                v_sb = vpool.tile([128, 4, H, 66], BF16, tag="v")  # [p, kt, h, hd+ones+pad]
                nc.gpsimd.memset(v_sb[:, :, :, 64:65], 1.0)
                for st in range(4):
                    vp_ps = ps_mm.tile([128, 512], F32, tag="mm")
                    for dc in range(4):
                        nc.tensor.matmul(
                            vp_ps[:],
                            xT[dc][:, st * 128 : (st + 1) * 128],
                            wqkv[:, dc, 1024:1536],
                            start=(dc == 0),
                            stop=(dc == 3),
                        )
                    nc.scalar.copy(
                        out=v_sb[:, st, :, 0:64],
                        in_=vp_ps[:].rearrange("p (h e) -> p h e", h=H),
                    )

                # ---- attention, head-pairs, software-pipelined ----
                y_sb = [
                    ypool.tile([128, 512], pdt, tag=f"y{qt}", name=f"ysb{qt}")
                    for qt in range(4)
                ]
                ynT = []

                def emit_scores(hg):
                    h0, h1 = 2 * hg, 2 * hg + 1
                    se = sepool.tile(
                        [128, 4, 2, 512], BF16, tag="se", name="se"
                    )  # [p, kt, hi, q]
                    for kt in range(4):
                        off = 128 * kt if kt < 3 else 256  # fp32r needs N>=256
                        offe = 128 * kt
                        stp = ps_att.tile([128, 1024], F32, tag="att", name="stp")
                        for hi, h in enumerate((h0, h1)):
                            mtq, poq = h // 2, 64 * (h % 2)
                            mtk, pok = 4 + h // 2, 64 * (h % 2)
                            nc.tensor.matmul(
                                stp[:, hi * 512 + off : hi * 512 + 512],
                                qkT[mtk][pok : pok + 64, kt * 128 : (kt + 1) * 128],
                                qkT[mtq][poq : poq + 64, off:512],
                                start=True,
                                stop=True,
                            )
                        nc.scalar.activation(
                            out=se[:, kt, :, offe:],
                            in_=stp[:].rearrange("p (hh q) -> p hh q", hh=2)[
                                :, :, offe:
                            ],
                            func=mybir.ActivationFunctionType.Exp,
                            scale=1.0 / np.sqrt(HD),
                        )
                        # mask the diagonal block (strict lower triangle -> 0)
                        nc.vector.tensor_mul(
                            out=se[:, kt, :, offe : offe + 128],
                            in0=se[:, kt, :, offe : offe + 128],
                            in1=tri2[:],
                        )
                    return se

                def emit_y(hg, se):
                    h0, h1 = 2 * hg, 2 * hg + 1
                    # y_aug[q, 65] = sum_kt p[kt].T @ V_aug[kt]
                    yp = ps_att.tile([128, 1024], F32, tag="att", name="yp")
                    for hi, h in enumerate((h0, h1)):
                        for qt in range(4):
                            base = hi * 512 + qt * 65
                            for kt in range(qt + 1):
                                nc.tensor.matmul(
                                    yp[:, base : base + 65],
                                    se[:, kt, hi, qt * 128 : (qt + 1) * 128],
                                    v_sb[:, kt, h, 0:65],
                                    start=(kt == 0),
                                    stop=(kt == qt),
                                )
                    # softmax denominators -> reciprocals
                    rs = rpool.tile([128, 2, 4], F32, tag="rs", name="rs")
                    nc.vector.reciprocal_approx_fast(
                        out=rs[:],
                        in_=yp[:].rearrange("p (hh q) -> p hh q", hh=2)[
                            :, :, 64:260:65
                        ],
                    )
                    # normalize + scatter into y_sb[qt][:, 64h:64h+64]
                    for hi, h in enumerate((h0, h1)):
                        for qt in range(4):
                            base = hi * 512 + qt * 65
                            nc.vector.tensor_scalar_mul(
                                y_sb[qt][:, 64 * h : 64 * h + 64],
                                yp[:, base : base + 64],
                                rs[:, hi, qt : qt + 1],
                            )
                    # yT transpose for the d-slice this head-pair completed
                    dc = hg
                    tp = ps_mm.tile([128, 512], F32, tag="mm", name="tpy")
                    for qt in range(4):
                        nc.tensor.transpose(
                            tp[:, qt * 128 : (qt + 1) * 128].bitcast(pdt),
                            y_sb[qt][:, dc * 128 : (dc + 1) * 128],
                            ident_y[:],
                        )
                    yt = ytpool.tile([128, 512], ytdt, tag=f"yt{dc}", name=f"yt{dc}")
                    nc.scalar.copy(out=yt[:], in_=tp[:])
                    ynT.append(yt)

                se_prev = emit_scores(0)
                for hg in range(4):
                    se_next = emit_scores(hg + 1) if hg + 1 < 4 else None
                    emit_y(hg, se_prev)
                    se_prev = se_next

                # ---- out = ynT.T @ wproj ----
                for qt in range(4):
                    op_ps = ps_mm.tile([128, 512], F32, tag="mm")
                    for dc in range(4):
                        lhs = ynT[dc][:, qt * 128 : (qt + 1) * 128]
                        rhs = wproj_c[:, dc, :]
                        nc.tensor.matmul(
                            op_ps[:], lhs, rhs, start=(dc == 0), stop=(dc == 3)
                        )
                    ob = opool.tile([128, 512], F32, tag="ob")
                    nc.vector.tensor_copy(out=ob[:], in_=op_ps[:])
                    nc.sync.dma_start(
                        out=out_d.ap()[b, qt * 128 : (qt + 1) * 128, :], in_=ob[:]
                    )

    nc.compile()
    return nc


def kernel(x, qkv_weight, proj_weight):
    if "nc" not in _cache:
        _cache["nc"] = build_nc()
    nc = _cache["nc"]
    in_maps = [
        {
            "x": np.ascontiguousarray(x[m], dtype=np.float32),
            "wqkv": np.ascontiguousarray(qkv_weight[m], dtype=np.float32),
            "wproj": np.ascontiguousarray(proj_weight[m], dtype=np.float32),
        }
        for m in range(M)
    ]
    res = bass_utils.run_bass_kernel_spmd(nc, in_maps, core_ids=list(range(N_CORES)))
    return np.stack([res.results[m]["out"] for m in range(M)]).astype(np.float32)


# revision 22
# speedup vs baseline: 1.1696x; 1.1696x over previous
"""Bass/Tile TRN2 kernel for per-model-batched causal self-attention.

Problem: x[M,B,S,D], qkv_w[M,D,3D], proj_w[M,D,D] -> out[M,B,S,D]
M=8 models sharded across 8 NeuronCores (embarrassingly parallel).

Per-core design (model m):
  xT      = PE-transpose(x_b)                       [D,S] f32
  qkT     = wqkv[:, :1024].T-proj via fp32r matmul  [1024,S] f32 (q^T,k^T rows)
  V       = x @ wqkv[:, 1024:]  (natural)           [S,512] -> bf16, +ones col
  st[k,q] = K @ Q^T  (fp32r, causal-trimmed)        PSUM f32
  p       = exp(st/8)  (ScalarE, bf16 out), diag blocks masked by tri01 mul
  y_aug   = p.T @ V_aug (bf16)  -> y[q,d] + softmax sums in col 64 (PSUM)
  y       = y_aug * (1/sums)  per-partition scalar  [S,D]
  ynT     = PE-transpose(y)                          [D,S]
  out     = ynT.T @ wproj (fp32r or bf16)
"""

import sys

if "/opt/trn_rl_repo" not in sys.path:
    sys.path.insert(0, "/opt/trn_rl_repo")

import numpy as np

import concourse.bass as bass
import concourse.mybir as mybir
import concourse.tile as tile
from concourse import bacc, bass_utils
from concourse.masks import make_identity, make_upper_triangular

M, B, S, D, H = 8, 4, 512, 512, 8
HD = D // H  # 64
F32 = mybir.dt.float32
F32R = mybir.dt.float32r
BF16 = mybir.dt.bfloat16

# --- knobs ---
PROJ_F32 = True  # final projection in fp32r (True) or bf16 (False)
N_CORES = 8

_cache = {}


def _r(ap):
    return ap.bitcast(F32R)


def build_nc(reps=1):
    nc = bacc.Bacc("TRN2", target_bir_lowering=False, debug=False)

    x_d = nc.dram_tensor("x", [B, S, D], F32, kind="ExternalInput")
    wqkv_d = nc.dram_tensor("wqkv", [D, 3 * D], F32, kind="ExternalInput")
    wproj_d = nc.dram_tensor("wproj", [D, D], F32, kind="ExternalInput")
    out_d = nc.dram_tensor("out", [B, S, D], F32, kind="ExternalOutput")

    pdt = F32R if PROJ_F32 else BF16
    ytdt = F32R if PROJ_F32 else BF16

    from contextlib import ExitStack, nullcontext

    with tile.TileContext(nc) as tc:
        with (
            tc.tile_pool(name="singles", bufs=1) as singles,
            tc.tile_pool(name="xp", bufs=2) as xpool,
            tc.tile_pool(name="xtp", bufs=3) as xtpool,
            tc.tile_pool(name="qk", bufs=2) as qkpool,
            tc.tile_pool(name="vp", bufs=2) as vpool,
            tc.tile_pool(name="se", bufs=3) as sepool,
            tc.tile_pool(name="yp", bufs=2) as ypool,
            tc.tile_pool(name="ytp", bufs=3) as ytpool,
            tc.tile_pool(name="op", bufs=3) as opool,
            tc.tile_pool(name="rp", bufs=4) as rpool,
            tc.tile_pool(name="ps_mm", bufs=2, space=bass.MemorySpace.PSUM) as ps_mm,
            tc.tile_pool(name="ps_att", bufs=3, space=bass.MemorySpace.PSUM) as ps_att,
        ):
          with tc.For_i(0, reps, 1) if reps > 1 else nullcontext():
            # ---- constants & weights (once) ----
            ident = singles.tile([128, 128], F32)
            make_identity(nc, ident[:])
            ident_r = singles.tile([128, 128], F32R)
            nc.vector.tensor_copy(out=ident_r[:], in_=ident[:])
            ident_y = ident_r
            if not PROJ_F32:
                ident_y = singles.tile([128, 128], BF16, name="identb")
                nc.vector.tensor_copy(out=ident_y[:], in_=ident[:])
            tri2 = singles.tile([128, 2, 128], BF16)  # upper-tri(incl diag) keep mask, x2
            make_upper_triangular(nc, tri2[:, 0, :], val=1.0, diag=True)
            nc.gpsimd.tensor_copy(out=tri2[:, 1, :], in_=tri2[:, 0, :])

            wqkv = singles.tile([128, 4, 3 * D], F32R)
            wproj = singles.tile([128, 4, D], F32R if PROJ_F32 else F32)
            wproj_c = wproj

            for b in range(B):
                # ---- load x_b ----
                x_sb = xpool.tile([128, 4, D], F32R, tag="x")  # [p, s_tile, d]
                nc.sync.dma_start(
                    out=x_sb[:], in_=x_d.ap().bitcast(F32R)[b].rearrange("(t p) d -> p t d", p=128)
                )
                if b == 0:
                    for dc in range(4):
                        nc.sync.dma_start(
                            out=wqkv[:, dc, :],
                            in_=wqkv_d.ap().bitcast(F32R)[dc * 128 : (dc + 1) * 128, :],
                        )
                    nc.sync.dma_start(
                        out=wproj[:],
                        in_=wproj_d.ap()
                        .bitcast(F32R if PROJ_F32 else F32)
                        .rearrange("(c p) o -> p c o", p=128),
                    )
                    if not PROJ_F32:
                        wproj_cc = singles.tile([128, 4, D], BF16, name="wprojc")
                        nc.vector.tensor_copy(out=wproj_cc[:], in_=wproj[:])
                        wproj_c = wproj_cc

                # ---- xT[d, s] via PE transpose ----
                xT = []
                for dc in range(4):
                    tp = ps_mm.tile([128, 512], F32, tag="mm")
                    for st in range(4):
                        nc.tensor.transpose(
                            tp[:, st * 128 : (st + 1) * 128].bitcast(F32R),
                            x_sb[:, st, dc * 128 : (dc + 1) * 128],
                            ident_r[:],
                        )
                    xt = xtpool.tile([128, 512], F32R, tag=f"xt{dc}")
                    nc.vector.tensor_copy(out=xt[:], in_=tp[:])
                    xT.append(xt)

                # ---- qkT[o, s] = wqkv[:, :1024].T @ xT  (fp32r) ----
                qkT = []
                for mt in range(8):
                    mp = ps_mm.tile([128, 512], F32, tag="mm")
                    for dc in range(4):
                        nc.tensor.matmul(
                            mp[:],
                            wqkv[:, dc, mt * 128 : (mt + 1) * 128],
                            xT[dc][:],
                            start=(dc == 0),
                            stop=(dc == 3),
                        )
                    qk = qkpool.tile([128, 512], F32R, tag=f"qk{mt}")
                    nc.vector.tensor_copy(out=qk[:], in_=mp[:])
                    qkT.append(qk)

                # ---- V[s, o'] natural (fp32r) + ones col -> bf16 V_aug ----
                v_sb = vpool.tile([128, 4, H, 66], BF16, tag="v")  # [p, kt, h, hd+ones+pad]
                nc.gpsimd.memset(v_sb[:, :, :, 64:65], 1.0)
                for st in range(4):
                    vp_ps = ps_mm.tile([128, 512], F32, tag="mm")
                    for dc in range(4):
                        nc.tensor.matmul(
                            vp_ps[:],
                            xT[dc][:, st * 128 : (st + 1) * 128],
                            wqkv[:, dc, 1024:1536],
                            start=(dc == 0),
                            stop=(dc == 3),
                        )
                    nc.scalar.copy(
                        out=v_sb[:, st, :, 0:64],
                        in_=vp_ps[:].rearrange("p (h e) -> p h e", h=H),
                    )

                # ---- attention, head-pairs, software-pipelined ----
                y_sb = [
                    ypool.tile([128, 512], pdt, tag=f"y{qt}", name=f"ysb{qt}")
                    for qt in range(4)
                ]
                ynT = []

                def emit_scores(hg):
                    h0, h1 = 2 * hg, 2 * hg + 1
                    se = sepool.tile(
                        [128, 4, 2, 512], BF16, tag="se", name="se"
                    )  # [p, kt, hi, q]
                    for kt in range(4):
                        off = 128 * kt if kt < 3 else 256  # fp32r needs N>=256
                        offe = 128 * kt
                        stp = ps_att.tile([128, 1024], F32, tag="att", name="stp")
                        for hi, h in enumerate((h0, h1)):
                            mtq, poq = h // 2, 64 * (h % 2)
                            mtk, pok = 4 + h // 2, 64 * (h % 2)
                            nc.tensor.matmul(
                                stp[:, hi * 512 + off : hi * 512 + 512],
                                qkT[mtk][pok : pok + 64, kt * 128 : (kt + 1) * 128],
                                qkT[mtq][poq : poq + 64, off:512],
                                start=True,
                                stop=True,
                            )
                        nc.scalar.activation(
                            out=se[:, kt, :, offe:],
                            in_=stp[:].rearrange("p (hh q) -> p hh q", hh=2)[
                                :, :, offe:
                            ],
                            func=mybir.ActivationFunctionType.Exp,
                            scale=1.0 / np.sqrt(HD),
                        )
                        # mask the diagonal block (strict lower triangle -> 0)
                        nc.vector.tensor_mul(
                            out=se[:, kt, :, offe : offe + 128],
                            in0=se[:, kt, :, offe : offe + 128],
                            in1=tri2[:],
                        )
                    return se

                def emit_y(hg, se):
                    h0, h1 = 2 * hg, 2 * hg + 1
                    # y_aug[q, 65] = sum_kt p[kt].T @ V_aug[kt]
                    yp = ps_att.tile([128, 1024], F32, tag="att", name="yp")
                    for hi, h in enumerate((h0, h1)):
                        for qt in range(4):
                            base = hi * 512 + qt * 65
                            for kt in range(qt + 1):
                                nc.tensor.matmul(
                                    yp[:, base : base + 65],
                                    se[:, kt, hi, qt * 128 : (qt + 1) * 128],
                                    v_sb[:, kt, h, 0:65],
                                    start=(kt == 0),
                                    stop=(kt == qt),
                                )
                    # softmax denominators -> reciprocals
                    rs = rpool.tile([128, 2, 4], F32, tag="rs", name="rs")
                    nc.vector.reciprocal_approx_fast(
                        out=rs[:],
                        in_=yp[:].rearrange("p (hh q) -> p hh q", hh=2)[
                            :, :, 64:260:65
                        ],
                    )
                    # normalize + scatter into y_sb[qt][:, 64h:64h+64]
                    for hi, h in enumerate((h0, h1)):
                        for qt in range(4):
                            base = hi * 512 + qt * 65
                            nc.vector.tensor_scalar_mul(
                                y_sb[qt][:, 64 * h : 64 * h + 64],
                                yp[:, base : base + 64],
                                rs[:, hi, qt : qt + 1],
                            )
                    # yT transpose for the d-slice this head-pair completed
                    dc = hg
                    tp = ps_mm.tile([128, 512], F32, tag="mm", name="tpy")
                    for qt in range(4):
                        nc.tensor.transpose(
                            tp[:, qt * 128 : (qt + 1) * 128].bitcast(pdt),
                            y_sb[qt][:, dc * 128 : (dc + 1) * 128],
                            ident_y[:],
                        )
                    yt = ytpool.tile([128, 512], ytdt, tag=f"yt{dc}", name=f"yt{dc}")
                    nc.scalar.copy(out=yt[:], in_=tp[:])
                    ynT.append(yt)

                se_prev = emit_scores(0)
                for hg in range(4):
                    se_next = emit_scores(hg + 1) if hg + 1 < 4 else None
                    emit_y(hg, se_prev)
                    se_prev = se_next

                # ---- out = ynT.T @ wproj ----
                for qt in range(4):
                    op_ps = ps_mm.tile([128, 512], F32, tag="mm")
                    for dc in range(4):
                        lhs = ynT[dc][:, qt * 128 : (qt + 1) * 128]
                        rhs = wproj_c[:, dc, :]
                        nc.tensor.matmul(
                            op_ps[:], lhs, rhs, start=(dc == 0), stop=(dc == 3)
                        )
                    ob = opool.tile([128, 512], F32, tag="ob")
                    nc.vector.tensor_copy(out=ob[:], in_=op_ps[:])
                    nc.sync.dma_start(
                        out=out_d.ap()[b, qt * 128 : (qt + 1) * 128, :], in_=ob[:]
                    )

    nc.compile()
    return nc


def kernel(x, qkv_weight, proj_weight):
    if "nc" not in _cache:
        _cache["nc"] = build_nc()
    nc = _cache["nc"]
    in_maps = [
        {
            "x": np.ascontiguousarray(x[m], dtype=np.float32),
            "wqkv": np.ascontiguousarray(qkv_weight[m], dtype=np.float32),
            "wproj": np.ascontiguousarray(proj_weight[m], dtype=np.float32),
        }
        for m in range(M)
    ]
    res = bass_utils.run_bass_kernel_spmd(nc, in_maps, core_ids=list(range(N_CORES)))
    return np.stack([res.results[m]["out"] for m in range(M)]).astype(np.float32)


# revision 23
# speedup vs baseline: 1.2395x; 1.0597x over previous
"""Bass/Tile TRN2 kernel (bf16 scores + fp32r projections) for per-model-batched causal self-attention.

Problem: x[M,B,S,D], qkv_w[M,D,3D], proj_w[M,D,D] -> out[M,B,S,D]
M=8 models sharded across 8 NeuronCores (embarrassingly parallel).

Per-core design (model m):
  xT      = PE-transpose(x_b)                       [D,S] f32
  qkT     = wqkv[:, :1024].T-proj via fp32r matmul  [1024,S] f32 (q^T,k^T rows)
  V       = x @ wqkv[:, 1024:]  (natural)           [S,512] -> bf16, +ones col
  st[k,q] = K @ Q^T  (fp32r, causal-trimmed)        PSUM f32
  p       = exp(st/8)  (ScalarE, bf16 out), diag blocks masked by tri01 mul
  y_aug   = p.T @ V_aug (bf16)  -> y[q,d] + softmax sums in col 64 (PSUM)
  y       = y_aug * (1/sums)  per-partition scalar  [S,D]
  ynT     = PE-transpose(y)                          [D,S]
  out     = ynT.T @ wproj (fp32r or bf16)
"""

import sys

if "/opt/trn_rl_repo" not in sys.path:
    sys.path.insert(0, "/opt/trn_rl_repo")

import numpy as np

import concourse.bass as bass
import concourse.mybir as mybir
import concourse.tile as tile
from concourse import bacc, bass_utils
from concourse.masks import make_identity, make_upper_triangular

M, B, S, D, H = 8, 4, 512, 512, 8
HD = D // H  # 64
F32 = mybir.dt.float32
F32R = mybir.dt.float32r
BF16 = mybir.dt.bfloat16

# --- knobs ---
PROJ_F32 = True  # final projection in fp32r (True) or bf16 (False)
N_CORES = 8

_cache = {}


def _r(ap):
    return ap.bitcast(F32R)


def build_nc(reps=1):
    nc = bacc.Bacc("TRN2", target_bir_lowering=False, debug=False)

    x_d = nc.dram_tensor("x", [B, S, D], F32, kind="ExternalInput")
    wqkv_d = nc.dram_tensor("wqkv", [D, 3 * D], F32, kind="ExternalInput")
    wproj_d = nc.dram_tensor("wproj", [D, D], F32, kind="ExternalInput")
    out_d = nc.dram_tensor("out", [B, S, D], F32, kind="ExternalOutput")

    pdt = F32R if PROJ_F32 else BF16
    ytdt = F32R if PROJ_F32 else BF16

    from contextlib import ExitStack, nullcontext

    with tile.TileContext(nc) as tc:
        with (
            tc.tile_pool(name="singles", bufs=1) as singles,
            tc.tile_pool(name="xp", bufs=2) as xpool,
            tc.tile_pool(name="xtp", bufs=3) as xtpool,
            tc.tile_pool(name="qk", bufs=2) as qkpool,
            tc.tile_pool(name="vp", bufs=2) as vpool,
            tc.tile_pool(name="se", bufs=3) as sepool,
            tc.tile_pool(name="yp", bufs=2) as ypool,
            tc.tile_pool(name="ytp", bufs=3) as ytpool,
            tc.tile_pool(name="op", bufs=3) as opool,
            tc.tile_pool(name="rp", bufs=4) as rpool,
            tc.tile_pool(name="ps_mm", bufs=2, space=bass.MemorySpace.PSUM) as ps_mm,
            tc.tile_pool(name="ps_att", bufs=3, space=bass.MemorySpace.PSUM) as ps_att,
        ):
          with tc.For_i(0, reps, 1) if reps > 1 else nullcontext():
            # ---- constants & weights (once) ----
            ident = singles.tile([128, 128], F32)
            make_identity(nc, ident[:])
            ident_r = singles.tile([128, 128], F32R)
            nc.vector.tensor_copy(out=ident_r[:], in_=ident[:])
            ident_y = ident_r
            if not PROJ_F32:
                ident_y = singles.tile([128, 128], BF16, name="identb")
                nc.vector.tensor_copy(out=ident_y[:], in_=ident[:])
            tri2 = singles.tile([128, 2, 128], BF16)  # upper-tri(incl diag) keep mask, x2
            make_upper_triangular(nc, tri2[:, 0, :], val=1.0, diag=True)
            nc.gpsimd.tensor_copy(out=tri2[:, 1, :], in_=tri2[:, 0, :])

            wqkv = singles.tile([128, 4, 3 * D], F32R)
            wproj = singles.tile([128, 4, D], F32R if PROJ_F32 else F32)
            wproj_c = wproj

            for b in range(B):
                # ---- load x_b ----
                x_sb = xpool.tile([128, 4, D], F32R, tag="x")  # [p, s_tile, d]
                nc.sync.dma_start(
                    out=x_sb[:], in_=x_d.ap().bitcast(F32R)[b].rearrange("(t p) d -> p t d", p=128)
                )
                if b == 0:
                    for dc in range(4):
                        nc.sync.dma_start(
                            out=wqkv[:, dc, :],
                            in_=wqkv_d.ap().bitcast(F32R)[dc * 128 : (dc + 1) * 128, :],
                        )
                    nc.sync.dma_start(
                        out=wproj[:],
                        in_=wproj_d.ap()
                        .bitcast(F32R if PROJ_F32 else F32)
                        .rearrange("(c p) o -> p c o", p=128),
                    )
                    if not PROJ_F32:
                        wproj_cc = singles.tile([128, 4, D], BF16, name="wprojc")
                        nc.vector.tensor_copy(out=wproj_cc[:], in_=wproj[:])
                        wproj_c = wproj_cc

                # ---- xT[d, s] via PE transpose ----
                xT = []
                for dc in range(4):
                    tp = ps_mm.tile([128, 512], F32, tag="mm")
                    for st in range(4):
                        nc.tensor.transpose(
                            tp[:, st * 128 : (st + 1) * 128].bitcast(F32R),
                            x_sb[:, st, dc * 128 : (dc + 1) * 128],
                            ident_r[:],
                        )
                    xt = xtpool.tile([128, 512], F32R, tag=f"xt{dc}")
                    nc.vector.tensor_copy(out=xt[:], in_=tp[:])
                    xT.append(xt)

                # ---- qkT[o, s] = wqkv[:, :1024].T @ xT  (fp32r) ----
                qkT = []
                for mt in range(8):
                    mp = ps_mm.tile([128, 512], F32, tag="mm")
                    for dc in range(4):
                        nc.tensor.matmul(
                            mp[:],
                            wqkv[:, dc, mt * 128 : (mt + 1) * 128],
                            xT[dc][:],
                            start=(dc == 0),
                            stop=(dc == 3),
                        )
                    qk = qkpool.tile([128, 512], BF16, tag=f"qk{mt}")
                    nc.vector.tensor_copy(out=qk[:], in_=mp[:])
                    qkT.append(qk)

                # ---- V[s, o'] natural (fp32r) + ones col -> bf16 V_aug ----
                v_sb = vpool.tile([128, 4, H, 66], BF16, tag="v")  # [p, kt, h, hd+ones+pad]
                nc.gpsimd.memset(v_sb[:, :, :, 64:65], 1.0)
                for st in range(4):
                    vp_ps = ps_mm.tile([128, 512], F32, tag="mm")
                    for dc in range(4):
                        nc.tensor.matmul(
                            vp_ps[:],
                            xT[dc][:, st * 128 : (st + 1) * 128],
                            wqkv[:, dc, 1024:1536],
                            start=(dc == 0),
                            stop=(dc == 3),
                        )
                    nc.scalar.copy(
                        out=v_sb[:, st, :, 0:64],
                        in_=vp_ps[:].rearrange("p (h e) -> p h e", h=H),
                    )

                # ---- attention, head-pairs, software-pipelined ----
                y_sb = [
                    ypool.tile([128, 512], pdt, tag=f"y{qt}", name=f"ysb{qt}")
                    for qt in range(4)
                ]
                ynT = []

                def emit_scores(hg):
                    h0, h1 = 2 * hg, 2 * hg + 1
                    se = sepool.tile(
                        [128, 4, 2, 512], BF16, tag="se", name="se"
                    )  # [p, kt, hi, q]
                    for kt in range(4):
                        off = 128 * kt  # scores are bf16: exact causal trim is fine
                        offe = off
                        stp = ps_att.tile([128, 1024], F32, tag="att", name="stp")
                        for hi, h in enumerate((h0, h1)):
                            mtq, poq = h // 2, 64 * (h % 2)
                            mtk, pok = 4 + h // 2, 64 * (h % 2)
                            nc.tensor.matmul(
                                stp[:, hi * 512 + off : hi * 512 + 512],
                                qkT[mtk][pok : pok + 64, kt * 128 : (kt + 1) * 128],
                                qkT[mtq][poq : poq + 64, off:512],
                                start=True,
                                stop=True,
                            )
                        nc.scalar.activation(
                            out=se[:, kt, :, offe:],
                            in_=stp[:].rearrange("p (hh q) -> p hh q", hh=2)[
                                :, :, offe:
                            ],
                            func=mybir.ActivationFunctionType.Exp,
                            scale=1.0 / np.sqrt(HD),
                        )
                        # mask the diagonal block (strict lower triangle -> 0)
                        nc.vector.tensor_mul(
                            out=se[:, kt, :, offe : offe + 128],
                            in0=se[:, kt, :, offe : offe + 128],
                            in1=tri2[:],
                        )
                    return se

                def emit_y(hg, se):
                    h0, h1 = 2 * hg, 2 * hg + 1
                    # y_aug[q, 65] = sum_kt p[kt].T @ V_aug[kt]
                    yp = ps_att.tile([128, 1024], F32, tag="att", name="yp")
                    for hi, h in enumerate((h0, h1)):
                        for qt in range(4):
                            base = hi * 512 + qt * 65
                            for kt in range(qt + 1):
                                nc.tensor.matmul(
                                    yp[:, base : base + 65],
                                    se[:, kt, hi, qt * 128 : (qt + 1) * 128],
                                    v_sb[:, kt, h, 0:65],
                                    start=(kt == 0),
                                    stop=(kt == qt),
                                )
                    # softmax denominators -> reciprocals
                    rs = rpool.tile([128, 2, 4], F32, tag="rs", name="rs")
                    nc.vector.reciprocal_approx_fast(
                        out=rs[:],
                        in_=yp[:].rearrange("p (hh q) -> p hh q", hh=2)[
                            :, :, 64:260:65
                        ],
                    )
                    # normalize + scatter into y_sb[qt][:, 64h:64h+64]
                    for hi, h in enumerate((h0, h1)):
                        for qt in range(4):
                            base = hi * 512 + qt * 65
                            nc.vector.tensor_scalar_mul(
                                y_sb[qt][:, 64 * h : 64 * h + 64],
                                yp[:, base : base + 64],
                                rs[:, hi, qt : qt + 1],
                            )
                    # yT transpose for the d-slice this head-pair completed
                    dc = hg
                    tp = ps_mm.tile([128, 512], F32, tag="mm", name="tpy")
                    for qt in range(4):
                        nc.tensor.transpose(
                            tp[:, qt * 128 : (qt + 1) * 128].bitcast(pdt),
                            y_sb[qt][:, dc * 128 : (dc + 1) * 128],
                            ident_y[:],
                        )
                    yt = ytpool.tile([128, 512], ytdt, tag=f"yt{dc}", name=f"yt{dc}")
                    nc.scalar.copy(out=yt[:], in_=tp[:])
                    ynT.append(yt)

                se_prev = emit_scores(0)
                for hg in range(4):
                    se_next = emit_scores(hg + 1) if hg + 1 < 4 else None
                    emit_y(hg, se_prev)
                    se_prev = se_next

                # ---- out = ynT.T @ wproj ----
                for qt in range(4):
                    op_ps = ps_mm.tile([128, 512], F32, tag="mm")
                    for dc in range(4):
                        lhs = ynT[dc][:, qt * 128 : (qt + 1) * 128]
                        rhs = wproj_c[:, dc, :]
                        nc.tensor.matmul(
                            op_ps[:], lhs, rhs, start=(dc == 0), stop=(dc == 3)
                        )
                    ob = opool.tile([128, 512], F32, tag="ob")
                    nc.vector.tensor_copy(out=ob[:], in_=op_ps[:])
                    nc.sync.dma_start(
                        out=out_d.ap()[b, qt * 128 : (qt + 1) * 128, :], in_=ob[:]
                    )

    nc.compile()
    return nc


def kernel(x, qkv_weight, proj_weight):
    if "nc" not in _cache:
        _cache["nc"] = build_nc()
    nc = _cache["nc"]
    in_maps = [
        {
            "x": np.ascontiguousarray(x[m], dtype=np.float32),
            "wqkv": np.ascontiguousarray(qkv_weight[m], dtype=np.float32),
            "wproj": np.ascontiguousarray(proj_weight[m], dtype=np.float32),
        }
        for m in range(M)
    ]
    res = bass_utils.run_bass_kernel_spmd(nc, in_maps, core_ids=list(range(N_CORES)))
    return np.stack([res.results[m]["out"] for m in range(M)]).astype(np.float32)


# revision 24
# speedup vs baseline: 1.2933x; 1.0434x over previous
"""Bass/Tile TRN2 kernel for per-model-batched causal self-attention.

Problem: x[M,B,S,D], qkv_w[M,D,3D], proj_w[M,D,D] -> out[M,B,S,D]
M=8 models sharded across 8 NeuronCores (embarrassingly parallel).

Per-core design (model m), per batch b:
  xT      = PE-transpose(x_b)  (f32r)               [D,S]
  qkT     = wqkv[:, :1024].T-proj (fp32r matmul)    [1024,S] -> bf16 (q^T,k^T rows)
  V       = x @ wqkv[:, 1024:] (fp32r)              [S,512] -> bf16, +ones col
  st[k,q] = K @ Q^T  (bf16, causal-trimmed,         PSUM f32
            head pairs auto-packed via tile_position)
  p       = exp(st/8)  (ScalarE, bf16 out), diag blocks masked by tri01 mul
  y_aug   = p.T @ V_aug (bf16)  -> y[q,d] + softmax sums in col 64 (PSUM)
  y       = y_aug * (1/sums)  per-partition scalar
  ynT     = PE-transpose(y) (f32r)                  [D,S]
  out     = ynT.T @ wproj (fp32r)

The next batch's load/transpose/projection groups are interleaved into the
attention loop (work queue) so the in-order PE has ready work while the
ScalarE exp chain runs.
"""

import sys

if "/opt/trn_rl_repo" not in sys.path:
    sys.path.insert(0, "/opt/trn_rl_repo")

from contextlib import nullcontext
from functools import partial

import numpy as np

import concourse.bass as bass
import concourse.mybir as mybir
import concourse.tile as tile
from concourse import bacc, bass_utils
from concourse.masks import make_identity, make_upper_triangular

M, B, S, D, H = 8, 4, 512, 512, 8
HD = D // H  # 64
F32 = mybir.dt.float32
F32R = mybir.dt.float32r
BF16 = mybir.dt.bfloat16

N_CORES = 8

_cache = {}


def build_nc(reps=1):
    nc = bacc.Bacc("TRN2", target_bir_lowering=False, debug=False)

    x_d = nc.dram_tensor("x", [B, S, D], F32, kind="ExternalInput")
    wqkv_d = nc.dram_tensor("wqkv", [D, 3 * D], F32, kind="ExternalInput")
    wproj_d = nc.dram_tensor("wproj", [D, D], F32, kind="ExternalInput")
    out_d = nc.dram_tensor("out", [B, S, D], F32, kind="ExternalOutput")

    with tile.TileContext(nc) as tc:
        with (
            tc.tile_pool(name="singles", bufs=1) as singles,
            tc.tile_pool(name="xp", bufs=2) as xpool,
            tc.tile_pool(name="xtp", bufs=3) as xtpool,
            tc.tile_pool(name="qk", bufs=2) as qkpool,
            tc.tile_pool(name="vp", bufs=2) as vpool,
            tc.tile_pool(name="se", bufs=3) as sepool,
            tc.tile_pool(name="yp", bufs=2) as ypool,
            tc.tile_pool(name="ytp", bufs=3) as ytpool,
            tc.tile_pool(name="op", bufs=3) as opool,
            tc.tile_pool(name="rp", bufs=4) as rpool,
            tc.tile_pool(name="ps_mm", bufs=2, space=bass.MemorySpace.PSUM) as ps_mm,
            tc.tile_pool(name="ps_att", bufs=2, space=bass.MemorySpace.PSUM) as ps_att,
        ):
          with tc.For_i(0, reps, 1) if reps > 1 else nullcontext():
            # ---- constants ----
            ident = singles.tile([128, 128], F32)
            make_identity(nc, ident[:])
            ident_r = singles.tile([128, 128], F32R)
            nc.vector.tensor_copy(out=ident_r[:], in_=ident[:])
            tri2 = singles.tile([128, 2, 128], BF16)  # keep-mask (k<=q), x2 heads
            make_upper_triangular(nc, tri2[:, 0, :], val=1.0, diag=True)
            nc.gpsimd.tensor_copy(out=tri2[:, 1, :], in_=tri2[:, 0, :])

            wqkv = singles.tile([128, 4, 3 * D], F32R)
            wproj = singles.tile([128, 4, D], F32R)

            state = {}

            # ---------- stage A (loads + projections), as schedulable groups ----
            def emit_load_x(b):
                x_sb = xpool.tile([128, 4, D], F32R, tag="x", name="xsb")
                nc.sync.dma_start(
                    out=x_sb[:],
                    in_=x_d.ap().bitcast(F32R)[b].rearrange("(t p) d -> p t d", p=128),
                )
                v_sb = vpool.tile([128, 4, H, 66], BF16, tag="v", name="vsb")
                nc.gpsimd.memset(v_sb[:, :, :, 64:65], 1.0)
                state[b] = {"x": x_sb, "xT": [], "qkT": [], "v": v_sb, "ynT": []}
                if b == 0:
                    for dc in range(4):
                        nc.sync.dma_start(
                            out=wqkv[:, dc, :],
                            in_=wqkv_d.ap().bitcast(F32R)[dc * 128 : (dc + 1) * 128, :],
                        )
                    nc.sync.dma_start(
                        out=wproj[:],
                        in_=wproj_d.ap()
                        .bitcast(F32R)
                        .rearrange("(c p) o -> p c o", p=128),
                    )

            def emit_xt_group(b, dc):
                st_ = state[b]
                tp = ps_mm.tile([128, 512], F32, tag="mm", name="tpx")
                for st in range(4):
                    nc.tensor.transpose(
                        tp[:, st * 128 : (st + 1) * 128].bitcast(F32R),
                        st_["x"][:, st, dc * 128 : (dc + 1) * 128],
                        ident_r[:],
                    )
                xt = xtpool.tile([128, 512], F32R, tag=f"xt{dc}", name=f"xt{dc}")
                nc.vector.tensor_copy(out=xt[:], in_=tp[:])
                st_["xT"].append(xt)

            def emit_qkt_group(b, mt):
                st_ = state[b]
                mp = ps_mm.tile([128, 512], F32, tag="mm", name="mp")
                for dc in range(4):
                    nc.tensor.matmul(
                        mp[:],
                        wqkv[:, dc, mt * 128 : (mt + 1) * 128],
                        st_["xT"][dc][:],
                        start=(dc == 0),
                        stop=(dc == 3),
                    )
                qk = qkpool.tile([128, 512], BF16, tag=f"qk{mt}", name=f"qk{mt}")
                nc.vector.tensor_copy(out=qk[:], in_=mp[:])
                st_["qkT"].append(qk)

            def emit_v_group(b, stt):
                st_ = state[b]
                vp_ps = ps_mm.tile([128, 512], F32, tag="mm", name="vp")
                for dc in range(4):
                    nc.tensor.matmul(
                        vp_ps[:],
                        st_["xT"][dc][:, stt * 128 : (stt + 1) * 128],
                        wqkv[:, dc, 1024:1536],
                        start=(dc == 0),
                        stop=(dc == 3),
                    )
                nc.scalar.copy(
                    out=st_["v"][:, stt, :, 0:64],
                    in_=vp_ps[:].rearrange("p (h e) -> p h e", h=H),
                )

            def proj_work(b):
                w = [partial(emit_load_x, b)]
                w += [partial(emit_xt_group, b, dc) for dc in range(4)]
                w += [partial(emit_qkt_group, b, mt) for mt in range(8)]
                w += [partial(emit_v_group, b, stt) for stt in range(4)]
                return w

            # ---------- attention ----------
            def emit_scores(b, hg):
                qkT = state[b]["qkT"]
                h0, h1 = 2 * hg, 2 * hg + 1
                se = sepool.tile([128, 4, 2, 512], BF16, tag="se", name="se")
                for kt in range(4):
                    off = 128 * kt
                    stp = ps_att.tile([128, 1024], F32, tag="att", name="stp")
                    for hi, h in enumerate((h0, h1)):
                        mtq, poq = h // 2, 64 * (h % 2)
                        mtk, pok = 4 + h // 2, 64 * (h % 2)
                        nc.tensor.matmul(
                            stp[:, hi * 512 + off : hi * 512 + 512],
                            qkT[mtk][pok : pok + 64, kt * 128 : (kt + 1) * 128],
                            qkT[mtq][poq : poq + 64, off:512],
                            start=True,
                            stop=True,
                        )
                    nc.scalar.activation(
                        out=se[:, kt, :, off:],
                        in_=stp[:].rearrange("p (hh q) -> p hh q", hh=2)[:, :, off:],
                        func=mybir.ActivationFunctionType.Exp,
                        scale=1.0 / np.sqrt(HD),
                    )
                    # mask the diagonal block (strict lower triangle -> 0)
                    nc.vector.tensor_mul(
                        out=se[:, kt, :, off : off + 128],
                        in0=se[:, kt, :, off : off + 128],
                        in1=tri2[:],
                    )
                return se

            def emit_y(b, hg, se, y_sb):
                st_ = state[b]
                h0, h1 = 2 * hg, 2 * hg + 1
                yp = ps_att.tile([128, 1024], F32, tag="att", name="yp")
                for hi, h in enumerate((h0, h1)):
                    for qt in range(4):
                        base = hi * 512 + qt * 65
                        for kt in range(qt + 1):
                            nc.tensor.matmul(
                                yp[:, base : base + 65],
                                se[:, kt, hi, qt * 128 : (qt + 1) * 128],
                                st_["v"][:, kt, h, 0:65],
                                start=(kt == 0),
                                stop=(kt == qt),
                            )
                rs = rpool.tile([128, 2, 4], F32, tag="rs", name="rs")
                nc.vector.reciprocal_approx_fast(
                    out=rs[:],
                    in_=yp[:].rearrange("p (hh q) -> p hh q", hh=2)[:, :, 64:260:65],
                )
                for hi, h in enumerate((h0, h1)):
                    for qt in range(4):
                        base = hi * 512 + qt * 65
                        nc.vector.tensor_scalar_mul(
                            y_sb[qt][:, 64 * h : 64 * h + 64],
                            yp[:, base : base + 64],
                            rs[:, hi, qt : qt + 1],
                        )
                # yT transpose for the d-slice this head-pair completed
                dc = hg
                tp = ps_mm.tile([128, 512], F32, tag="mm", name="tpy")
                for qt in range(4):
                    nc.tensor.transpose(
                        tp[:, qt * 128 : (qt + 1) * 128].bitcast(F32R),
                        y_sb[qt][:, dc * 128 : (dc + 1) * 128],
                        ident_r[:],
                    )
                yt = ytpool.tile([128, 512], F32R, tag=f"yt{dc}", name=f"yt{dc}")
                nc.scalar.copy(out=yt[:], in_=tp[:])
                st_["ynT"].append(yt)

            # ---------- main schedule ----------
            for f in proj_work(0):
                f()
            for b in range(B):
                queue = proj_work(b + 1) if b + 1 < B else []
                y_sb = [
                    ypool.tile([128, 512], F32R, tag=f"y{qt}", name=f"ysb{qt}")
                    for qt in range(4)
                ]
                se_prev = emit_scores(b, 0)
                for hg in range(4):
                    se_next = emit_scores(b, hg + 1) if hg + 1 < 4 else None
                    # fill PE while ScalarE runs the exp chain for this hg
                    for _ in range(5):
                        if queue:
                            queue.pop(0)()
                    emit_y(b, hg, se_prev, y_sb)
                    se_prev = se_next
                while queue:
                    queue.pop(0)()

                # ---- out = ynT.T @ wproj (fp32r) ----
                ynT = state[b]["ynT"]
                for qt in range(4):
                    op_ps = ps_mm.tile([128, 512], F32, tag="mm", name="op")
                    for dc in range(4):
                        nc.tensor.matmul(
                            op_ps[:],
                            ynT[dc][:, qt * 128 : (qt + 1) * 128],
                            wproj[:, dc, :],
                            start=(dc == 0),
                            stop=(dc == 3),
                        )
                    ob = opool.tile([128, 512], F32, tag="ob", name="ob")
                    nc.vector.tensor_copy(out=ob[:], in_=op_ps[:])
                    nc.sync.dma_start(
                        out=out_d.ap()[b, qt * 128 : (qt + 1) * 128, :], in_=ob[:]
                    )
                del state[b]

    nc.compile()
    return nc


def kernel(x, qkv_weight, proj_weight):
    if "nc" not in _cache:
        _cache["nc"] = build_nc()
    nc = _cache["nc"]
    in_maps = [
        {
            "x": np.ascontiguousarray(x[m], dtype=np.float32),
            "wqkv": np.ascontiguousarray(qkv_weight[m], dtype=np.float32),
            "wproj": np.ascontiguousarray(proj_weight[m], dtype=np.float32),
        }
        for m in range(M)
    ]
    res = bass_utils.run_bass_kernel_spmd(nc, in_maps, core_ids=list(range(N_CORES)))
    return np.stack([res.results[m]["out"] for m in range(M)]).astype(np.float32)
